# revision 13
# baseline (speedup 1.0000x reference)
"""Trainium2 Bass kernel for multi-head attention (nn_Attention).

Problem: x[8, 32, 32, 768] -> MHA(12 heads, d=64) -> out[8, 32, 32, 768].

Sharding: pure data parallel. Batch B=8 maps 1:1 onto the 8 NeuronCores;
weights are replicated. No collectives.

Per-core algorithm (N=1024 tokens, C=768):
  1. DMA x, qkv_w natural; PE-transpose to feature-major xT[c,n], WT[c,o]
     (contraction dim must live on SBUF partitions).
  2. qT/kT (feature-major) = WT.T @ xT;  V (token-major) = xT.T @ WT_v.
     V is stored bf16 with a ones-column appended per head.
  3. Per head: S^T[j,i] = kT.T @ qT (K=64 matmuls, two heads packed into
     the 128-row PE array via tile_position).  E = exp(S^T/8) via ACT
     directly out of PSUM (no max-subtraction needed: scores ~ N(0,1)).
  4. PV: out^T[d,i] (+ denominator row, from the ones-column) =
     [V|1].T @ E — no transpose of the probability matrix anywhere.
  5. Normalize columns by 1/denom (PE ones-broadcast + DVE multiply),
     accumulate feature-major O^T.
  6. out = O^T.T @ PwT + proj_b, DMA out per token tile.

Matmuls run as float32r (full PE rate at free-dim >= 256); E/V in bf16.
"""

import os
import sys

for _p in ("/opt/trn_rl_repo",):
    if _p not in sys.path:
        sys.path.insert(0, _p)

import numpy as np

import concourse.bass as bass
from concourse import bacc
import concourse.mybir as mybir
from concourse.masks import make_identity
from concourse.tile import TileContext

F32 = mybir.dt.float32
F32R = mybir.dt.float32r
BF16 = mybir.dt.bfloat16

P = 128
C = 768            # model dim
CT = C // P        # 6 c-tiles
N = 1024           # tokens per batch element
NT = N // P        # 8 token tiles
HEADS = 12
D = 64
OQK = 2 * C        # 1536 rows of q+k features
OTQK = OQK // P    # 12
OT3 = 3 * C // P   # 18 qkv_w row tiles
SCALE = D ** -0.5  # 0.125


def build_nc() -> bass.Bass:
    nc = bacc.Bacc(None, target_bir_lowering=False)
    x_d = nc.declare_dram_parameter("x", [N, C], F32, isOutput=False)
    qkvw_d = nc.declare_dram_parameter("qkv_w", [3 * C, C], F32, isOutput=False)
    qkvb_d = nc.declare_dram_parameter("qkv_b", [3 * C], F32, isOutput=False)
    projw_d = nc.declare_dram_parameter("proj_w", [C, C], F32, isOutput=False)
    projb_d = nc.declare_dram_parameter("proj_b", [C], F32, isOutput=False)
    out_d = nc.declare_dram_parameter("out", [N, C], F32, isOutput=True)

    with TileContext(nc) as tc:
        with (
            tc.tile_pool(name="const", bufs=1) as cpool,
            tc.tile_pool(name="load", bufs=3) as lpool,
            tc.tile_pool(name="qk", bufs=1) as qkpool,
            tc.tile_pool(name="v", bufs=1) as vpool,
            tc.tile_pool(name="otp", bufs=1) as otpool,
            tc.tile_pool(name="psA", bufs=4, space="PSUM") as psA,
            tc.tile_pool(name="psS", bufs=2, space="PSUM") as psS,
        ):
            ident = cpool.tile([P, P], F32, tag="ident")
            make_identity(nc, ident)
            ones_st = cpool.tile([1, P], F32, tag="ones_st")
            nc.gpsimd.memset(ones_st, 1.0)
            ones_row = cpool.tile([1, P], F32R, tag="ones")
            nc.vector.tensor_copy(ones_row, ones_st)

            # Biases. q/k bias is applied per-partition (feature-major);
            # v/proj biases are applied by seeding the PSUM accumulation
            # with a ones-outer-product matmul (free-dim broadcast).
            bqk = cpool.tile([P, OTQK], F32, tag="bqk")
            nc.sync.dma_start(bqk, qkvb_d[0:OQK].rearrange("(t p) -> p t", p=P))
            bv_st = cpool.tile([1, C], F32, tag="bv_st")
            nc.sync.dma_start(bv_st, qkvb_d[None, OQK : 3 * C])
            bv = cpool.tile([1, C], F32R, tag="bv")
            nc.vector.tensor_copy(bv, bv_st)
            pb_st = cpool.tile([1, C], F32, tag="pb_st")
            nc.sync.dma_start(pb_st, projb_d[None, :])
            pb = cpool.tile([1, C], F32R, tag="pb")
            nc.vector.tensor_copy(pb, pb_st)

            # Persistent activations
            qkT = qkpool.tile([P, OTQK, N], F32R, tag="qkT")      # q,k feature-major
            V = vpool.tile([P, NT, HEADS, D + 1], BF16, tag="V")  # token-major + ones col
            OT = otpool.tile([P, CT, N], F32R, tag="OT")           # attn out, feature-major

            nc.gpsimd.memset(V[:, :, :, D], 1.0)

            with (
                tc.tile_pool(name="xT", bufs=1) as xtpool,
                tc.tile_pool(name="wT", bufs=1) as wtpool,
            ):
                xT = xtpool.tile([P, CT, N], F32R, tag="xT")
                WT = wtpool.tile([P, CT, 3 * C], F32R, tag="WT")

                # ---- load + transpose x ----
                for nt in range(NT):
                    xt = lpool.tile([P, C], F32, tag="ld")
                    nc.sync.dma_start(xt, x_d[nt * P : (nt + 1) * P, :])
                    for ct in range(CT):
                        ps = psA.tile([P, 512], F32, tag="psA")
                        nc.tensor.transpose(ps[:, 0:P], xt[:, ct * P : (ct + 1) * P], ident)
                        nc.vector.tensor_copy(xT[:, ct, nt * P : (nt + 1) * P], ps[:, 0:P])

                # ---- load + transpose qkv_w ----
                for ot in range(OT3):
                    wt = lpool.tile([P, C], F32, tag="ld")
                    nc.sync.dma_start(wt, qkvw_d[ot * P : (ot + 1) * P, :])
                    for ct in range(CT):
                        ps = psA.tile([P, 512], F32, tag="psA")
                        nc.tensor.transpose(ps[:, 0:P], wt[:, ct * P : (ct + 1) * P], ident)
                        nc.vector.tensor_copy(WT[:, ct, ot * P : (ot + 1) * P], ps[:, 0:P])

                # ---- q,k projection (feature-major) ----
                for ot in range(OTQK):
                    for ic in range(2):
                        ps = psA.tile([P, 512], F32, tag="psA")
                        for ct in range(CT):
                            nc.tensor.matmul(
                                ps,
                                WT[:, ct, ot * P : (ot + 1) * P],
                                xT[:, ct, ic * 512 : (ic + 1) * 512],
                                start=(ct == 0),
                                stop=(ct == CT - 1),
                            )
                        nc.vector.tensor_scalar_add(
                            qkT[:, ot, ic * 512 : (ic + 1) * 512], ps, bqk[:, ot : ot + 1]
                        )

                # ---- v projection (token-major, bias-seeded) ----
                for nt in range(NT):
                    for o0, ow in ((0, 512), (512, 256)):
                        ps = psA.tile([P, 512], F32, tag="psA")
                        pss = ps[:, :ow]
                        nc.tensor.matmul(
                            pss, ones_row, bv[:, o0 : o0 + ow], start=True, stop=False
                        )
                        for ct in range(CT):
                            nc.tensor.matmul(
                                pss,
                                xT[:, ct, nt * P : (nt + 1) * P],
                                WT[:, ct, OQK + o0 : OQK + o0 + ow],
                                start=False,
                                stop=(ct == CT - 1),
                            )
                        h0 = o0 // D
                        nc.vector.tensor_copy(
                            V[:, nt, h0 : h0 + ow // D, 0:D],
                            pss.rearrange("p (h d) -> p h d", d=D),
                        )

            with (
                tc.tile_pool(name="pw", bufs=1) as pwpool,
                tc.tile_pool(name="e", bufs=3) as epool,
                tc.tile_pool(name="rec", bufs=2) as rpool,
                tc.tile_pool(name="outs", bufs=2) as outpool,
            ):
                PwT = pwpool.tile([P, CT, C], F32R, tag="PwT")

                # ---- load + transpose proj_w (overlaps with head phase) ----
                for ot in range(CT):
                    wt = lpool.tile([P, C], F32, tag="ld")
                    nc.sync.dma_start(wt, projw_d[ot * P : (ot + 1) * P, :])
                    for ct in range(CT):
                        ps = psA.tile([P, 512], F32, tag="psA")
                        nc.tensor.transpose(ps[:, 0:P], wt[:, ct * P : (ct + 1) * P], ident)
                        nc.vector.tensor_copy(PwT[:, ct, ot * P : (ot + 1) * P], ps[:, 0:P])

                # ---- attention heads, processed as packed pairs ----
                for pair in range(HEADS // 2):
                    E0 = epool.tile([P, NT, N], BF16, tag="E", name="E0")
                    E1 = epool.tile([P, NT, N], BF16, tag="E", name="E1")
                    Es = (E0, E1)
                    for jt in range(NT):
                        ps0 = psS.tile([P, N], F32, tag="psS", name="ps0")
                        ps1 = psS.tile([P, N], F32, tag="psS", name="ps1")
                        pss = (ps0, ps1)
                        for half in range(2):
                            lo, hi = half * D, half * D + D
                            for ic in range(2):
                                nc.tensor.matmul(
                                    pss[half][:, ic * 512 : (ic + 1) * 512],
                                    qkT[lo:hi, OTQK // 2 + pair, jt * P : (jt + 1) * P],
                                    qkT[lo:hi, pair, ic * 512 : (ic + 1) * 512],
                                    start=True,
                                    stop=True,
                                    tile_position=(half * D, 0),
                                )
                        for half in range(2):
                            nc.scalar.activation(
                                Es[half][:, jt, :],
                                pss[half],
                                mybir.ActivationFunctionType.Exp,
                                scale=SCALE,
                            )
                    for half in range(2):
                        h = 2 * pair + half
                        E = Es[half]
                        for ic in range(2):
                            pspv = psA.tile([P, 512], F32, tag="psA")
                            for jt in range(NT):
                                nc.tensor.matmul(
                                    pspv[0 : D + 1, :],
                                    V[:, jt, h, :],
                                    E[:, jt, ic * 512 : (ic + 1) * 512],
                                    start=(jt == 0),
                                    stop=(jt == NT - 1),
                                )
                            rec = rpool.tile([1, 512], F32R, tag="rec")
                            with nc.allow_low_precision(reason="denom recip feeds fp32r broadcast matmul"):
                                nc.vector.reciprocal(rec, pspv[D : D + 1, :])
                            psb = psA.tile([P, 512], F32, tag="psA")
                            nc.tensor.matmul(psb, ones_row, rec, start=True, stop=True)
                            bcast = rpool.tile([D, 512], F32, tag="bc")
                            nc.vector.tensor_copy(bcast, psb[0:D, :])
                            nc.vector.tensor_mul(
                                OT[half * D : half * D + D, h // 2, ic * 512 : (ic + 1) * 512],
                                pspv[0:D, :],
                                bcast,
                            )

                # ---- output projection ----
                for it in range(NT):
                    outt = outpool.tile([P, C], F32, tag="out")
                    for o0, ow in ((0, 512), (512, 256)):
                        ps = psA.tile([P, 512], F32, tag="psA")
                        pss = ps[:, :ow]
                        nc.tensor.matmul(
                            pss, ones_row, pb[:, o0 : o0 + ow], start=True, stop=False
                        )
                        for ct in range(CT):
                            nc.tensor.matmul(
                                pss,
                                OT[:, ct, it * P : (it + 1) * P],
                                PwT[:, ct, o0 : o0 + ow],
                                start=False,
                                stop=(ct == CT - 1),
                            )
                        nc.vector.tensor_copy(outt[:, o0 : o0 + ow], pss)
                    nc.sync.dma_start(out_d[it * P : (it + 1) * P, :], outt)

    nc.compile()
    return nc


_NC_CACHE = None


def _get_nc():
    global _NC_CACHE
    if _NC_CACHE is None:
        _NC_CACHE = build_nc()
    return _NC_CACHE


def run(inputs, trace=False, tmpdir=None):
    """Run on 8 NeuronCores; returns (out[8,32,32,768], BassKernelResults)."""
    from concourse.bass_utils import run_bass_kernel_spmd

    x = np.asarray(inputs["x"], dtype=np.float32)
    B, H, W, Cc = x.shape
    xf = np.ascontiguousarray(x.reshape(B, H * W, Cc))
    qkv_w = np.ascontiguousarray(np.asarray(inputs["qkv_w"], dtype=np.float32))
    qkv_b = np.ascontiguousarray(np.asarray(inputs["qkv_b"], dtype=np.float32))
    proj_w = np.ascontiguousarray(np.asarray(inputs["proj_w"], dtype=np.float32))
    proj_b = np.ascontiguousarray(np.asarray(inputs["proj_b"], dtype=np.float32))

    nc = _get_nc()
    in_maps = [
        {
            "x": xf[b],
            "qkv_w": qkv_w,
            "qkv_b": qkv_b,
            "proj_w": proj_w,
            "proj_b": proj_b,
        }
        for b in range(B)
    ]
    res = run_bass_kernel_spmd(nc, in_maps, list(range(B)), trace=trace, tmpdir=tmpdir)
    out = np.stack([res.results[b]["out"] for b in range(B)])
    return out.reshape(B, H, W, Cc).astype(np.float32), res


def kernel(x, qkv_w, qkv_b, proj_w, proj_b):
    out, _ = run(
        {
            "x": x,
            "qkv_w": qkv_w,
            "qkv_b": qkv_b,
            "proj_w": proj_w,
            "proj_b": proj_b,
        }
    )
    return out


# revision 19
# speedup vs baseline: 1.1401x; 1.1401x over previous
"""Trainium2 Bass kernel for multi-head attention (nn_Attention).

Problem: x[8, 32, 32, 768] -> MHA(12 heads, d=64) -> out[8, 32, 32, 768].

Sharding: pure data parallel. Batch B=8 maps 1:1 onto the 8 NeuronCores;
weights are replicated. No collectives.

Per-core algorithm (N=1024 tokens, C=768), all matmuls in bf16 with fp32
PSUM accumulation (bf16 moving operand allows free-dim 1024, halving the
matmul/LDWEIGHTS count vs fp32):
  1. DMA x, qkv_w natural; DVE-cast to bf16; PE-transpose to feature-major
     xT[c,n], WT[c,o] (contraction dim must live on SBUF partitions).
  2. qT/kT (feature-major) = WT.T @ xT;  V (token-major) = xT.T @ WT_v,
     stored with a ones-column appended per head.
  3. Per head pair: S^T[j,i] = kT.T @ qT (K=64, two heads packed into the
     128-row PE array via tile_position).  E = exp(S^T/8) via ACT directly
     out of PSUM (no max-subtraction needed: scores ~ N(0,1)).
  4. PV: out^T[d,i] (+ denominator row, from the ones-column) =
     [V|1].T @ E — no transpose of the probability matrix anywhere.
  5. Normalize columns by 1/denom (fast-approx reciprocal, PE
     ones-broadcast, DVE multiply), accumulate feature-major O^T.
  6. out = O^T.T @ PwT + proj_b, DMA out per token tile.
"""

import os
import sys

for _p in ("/opt/trn_rl_repo",):
    if _p not in sys.path:
        sys.path.insert(0, _p)

import numpy as np

import concourse.bass as bass
from concourse import bacc
import concourse.mybir as mybir
from concourse.masks import make_identity
from concourse.tile import TileContext

F32 = mybir.dt.float32
F32R = mybir.dt.float32r
BF16 = mybir.dt.bfloat16

P = 128
C = 768            # model dim
CT = C // P        # 6 c-tiles
N = 1024           # tokens per batch element
NT = N // P        # 8 token tiles
HEADS = 12
D = 64
OQK = 2 * C        # 1536 rows of q+k features
OTQK = OQK // P    # 12
OT3 = 3 * C // P   # 18 qkv_w row tiles
SCALE = D ** -0.5  # 0.125


def build_nc() -> bass.Bass:
    nc = bacc.Bacc(None, target_bir_lowering=False)
    x_d = nc.declare_dram_parameter("x", [N, C], F32, isOutput=False)
    qkvw_d = nc.declare_dram_parameter("qkv_w", [3 * C, C], F32, isOutput=False)
    qkvb_d = nc.declare_dram_parameter("qkv_b", [3 * C], F32, isOutput=False)
    projw_d = nc.declare_dram_parameter("proj_w", [C, C], F32, isOutput=False)
    projb_d = nc.declare_dram_parameter("proj_b", [C], F32, isOutput=False)
    out_d = nc.declare_dram_parameter("out", [N, C], F32, isOutput=True)

    with TileContext(nc) as tc:
        with (
            tc.tile_pool(name="const", bufs=1) as cpool,
            tc.tile_pool(name="load", bufs=3) as lpool,
            tc.tile_pool(name="ldb", bufs=3) as lbpool,
            tc.tile_pool(name="qk", bufs=1) as qkpool,
            tc.tile_pool(name="v", bufs=1) as vpool,
            tc.tile_pool(name="otp", bufs=1) as otpool,
            tc.tile_pool(name="ps", bufs=4, space="PSUM") as pspool,
        ):
            ident = cpool.tile([P, P], F32, tag="ident")
            make_identity(nc, ident)
            ones_st = cpool.tile([1, P], F32, tag="ones_st")
            nc.gpsimd.memset(ones_st, 1.0)
            ones_row = cpool.tile([1, P], BF16, tag="ones")
            nc.vector.tensor_copy(ones_row, ones_st)
            ones_r = cpool.tile([1, P], F32R, tag="ones_r")
            nc.vector.tensor_copy(ones_r, ones_st)

            # Biases. q/k bias is applied per-partition (feature-major);
            # v/proj biases are applied by seeding the PSUM accumulation
            # with a ones-outer-product matmul (free-dim broadcast).
            bqk = cpool.tile([P, OTQK], F32, tag="bqk")
            nc.sync.dma_start(bqk, qkvb_d[0:OQK].rearrange("(t p) -> p t", p=P))
            bv_st = cpool.tile([1, C], F32, tag="bv_st")
            nc.sync.dma_start(bv_st, qkvb_d[None, OQK : 3 * C])
            bv = cpool.tile([1, C], BF16, tag="bv")
            nc.vector.tensor_copy(bv, bv_st)
            pb_st = cpool.tile([1, C], F32, tag="pb_st")
            nc.sync.dma_start(pb_st, projb_d[None, :])
            pb = cpool.tile([1, C], BF16, tag="pb")
            nc.vector.tensor_copy(pb, pb_st)

            # Persistent activations
            qkT = qkpool.tile([P, OTQK, N], BF16, tag="qkT")      # q,k feature-major
            V = vpool.tile([P, NT, HEADS, D + 1], BF16, tag="V")  # token-major + ones col
            OT = otpool.tile([P, CT, N], BF16, tag="OT")          # attn out, feature-major

            nc.gpsimd.memset(V[:, :, :, D], 1.0)

            def load_cast_transpose(dram_row_tile, dest, dest_block):
                """DMA one [128, C] row tile, fp32 PE-transpose the six
                [128,128] blocks, casting to bf16 on the PSUM->SBUF copy."""
                st = lpool.tile([P, C], F32, tag="ld")
                nc.sync.dma_start(st, dram_row_tile)
                for ct in range(CT):
                    ps = pspool.tile([P, P], F32, tag="ps")
                    nc.tensor.transpose(ps, st[:, ct * P : (ct + 1) * P], ident)
                    nc.vector.tensor_copy(
                        dest[:, ct, dest_block * P : (dest_block + 1) * P], ps
                    )

            with (
                tc.tile_pool(name="xT", bufs=1) as xtpool,
                tc.tile_pool(name="wT", bufs=1) as wtpool,
            ):
                xT = xtpool.tile([P, CT, N], BF16, tag="xT")
                WT = wtpool.tile([P, CT, 3 * C], BF16, tag="WT")

                for nt in range(NT):
                    load_cast_transpose(x_d[nt * P : (nt + 1) * P, :], xT, nt)
                for ot in range(OT3):
                    load_cast_transpose(qkvw_d[ot * P : (ot + 1) * P, :], WT, ot)

                # ---- q,k projection (feature-major) ----
                for ot in range(OTQK):
                    ps = pspool.tile([P, N], F32, tag="ps")
                    for ic in range(2):
                        for ct in range(CT):
                            nc.tensor.matmul(
                                ps[:, ic * 512 : (ic + 1) * 512],
                                WT[:, ct, ot * P : (ot + 1) * P],
                                xT[:, ct, ic * 512 : (ic + 1) * 512],
                                start=(ct == 0),
                                stop=(ct == CT - 1),
                            )
                    nc.vector.tensor_scalar_add(qkT[:, ot, :], ps, bqk[:, ot : ot + 1])

                # ---- v projection (token-major, bias-seeded), free dim 768 ----
                for nt in range(NT):
                    ps = pspool.tile([P, N], F32, tag="ps")
                    for o0, ow in ((0, 512), (512, 256)):
                        pss = ps[:, o0 : o0 + ow]
                        nc.tensor.matmul(
                            pss, ones_row, bv[:, o0 : o0 + ow], start=True, stop=False
                        )
                        for ct in range(CT):
                            nc.tensor.matmul(
                                pss,
                                xT[:, ct, nt * P : (nt + 1) * P],
                                WT[:, ct, OQK + o0 : OQK + o0 + ow],
                                start=False,
                                stop=(ct == CT - 1),
                            )
                    nc.vector.tensor_copy(
                        V[:, nt, :, 0:D],
                        ps[:, :C].rearrange("p (h d) -> p h d", d=D),
                    )

            with (
                tc.tile_pool(name="pw", bufs=1) as pwpool,
                tc.tile_pool(name="e", bufs=3) as epool,
                tc.tile_pool(name="rec", bufs=2) as rpool,
                tc.tile_pool(name="outs", bufs=2) as outpool,
            ):
                PwT = pwpool.tile([P, CT, C], BF16, tag="PwT")
                for ot in range(CT):
                    load_cast_transpose(projw_d[ot * P : (ot + 1) * P, :], PwT, ot)

                # ---- attention heads, processed as packed pairs ----
                for pair in range(HEADS // 2):
                    E0 = epool.tile([P, NT, N], BF16, tag="E", name="E0")
                    E1 = epool.tile([P, NT, N], BF16, tag="E", name="E1")
                    Es = (E0, E1)
                    for jt in range(NT):
                        for half in range(2):
                            lo, hi = half * D, half * D + D
                            ps = pspool.tile([P, N], F32, tag="ps", name="ps_s")
                            for ic in range(2):
                                nc.tensor.matmul(
                                    ps[:, ic * 512 : (ic + 1) * 512],
                                    qkT[lo:hi, OTQK // 2 + pair, jt * P : (jt + 1) * P],
                                    qkT[lo:hi, pair, ic * 512 : (ic + 1) * 512],
                                    start=True,
                                    stop=True,
                                    tile_position=(half * D, 0),
                                )
                            nc.scalar.activation(
                                Es[half][:, jt, :],
                                ps,
                                mybir.ActivationFunctionType.Exp,
                                scale=SCALE,
                            )
                    for half in range(2):
                        h = 2 * pair + half
                        E = Es[half]
                        pspv = pspool.tile([P, N], F32, tag="ps", name="ps_pv")
                        for ic in range(2):
                            for jt in range(NT):
                                nc.tensor.matmul(
                                    pspv[0 : D + 1, ic * 512 : (ic + 1) * 512],
                                    V[:, jt, h, :],
                                    E[:, jt, ic * 512 : (ic + 1) * 512],
                                    start=(jt == 0),
                                    stop=(jt == NT - 1),
                                )
                        den_sb = rpool.tile([1, N], F32, tag="den_sb")
                        nc.vector.tensor_copy(den_sb, pspv[D : D + 1, :])
                        rec_st = rpool.tile([1, N], F32, tag="rec_st")
                        nc.vector.reciprocal_approx_fast(rec_st, den_sb)
                        rec = rpool.tile([1, N], F32R, tag="rec")
                        nc.vector.tensor_copy(rec, rec_st)
                        psb = pspool.tile([P, N], F32, tag="ps", name="ps_bc")
                        for ic in range(2):
                            nc.tensor.matmul(
                                psb[:, ic * 512 : (ic + 1) * 512],
                                ones_r,
                                rec[:, ic * 512 : (ic + 1) * 512],
                                start=True,
                                stop=True,
                            )
                        bcast = rpool.tile([D, N], BF16, tag="bc")
                        nc.vector.tensor_copy(bcast, psb[0:D, :])
                        nc.vector.tensor_mul(
                            OT[half * D : half * D + D, h // 2, :],
                            pspv[0:D, :],
                            bcast,
                        )

                # ---- output projection, free dim 768 ----
                for it in range(NT):
                    outt = outpool.tile([P, C], F32, tag="out")
                    ps = pspool.tile([P, N], F32, tag="ps", name="ps_o")
                    for o0, ow in ((0, 512), (512, 256)):
                        pss = ps[:, o0 : o0 + ow]
                        nc.tensor.matmul(
                            pss, ones_row, pb[:, o0 : o0 + ow], start=True, stop=False
                        )
                        for ct in range(CT):
                            nc.tensor.matmul(
                                pss,
                                OT[:, ct, it * P : (it + 1) * P],
                                PwT[:, ct, o0 : o0 + ow],
                                start=False,
                                stop=(ct == CT - 1),
                            )
                    nc.vector.tensor_copy(outt, ps[:, :C])
                    nc.sync.dma_start(out_d[it * P : (it + 1) * P, :], outt)

    nc.compile()
    return nc


_NC_CACHE = None


def _get_nc():
    global _NC_CACHE
    if _NC_CACHE is None:
        _NC_CACHE = build_nc()
    return _NC_CACHE


def run(inputs, trace=False, tmpdir=None):
    """Run on 8 NeuronCores; returns (out[8,32,32,768], BassKernelResults)."""
    from concourse.bass_utils import run_bass_kernel_spmd

    x = np.asarray(inputs["x"], dtype=np.float32)
    B, H, W, Cc = x.shape
    xf = np.ascontiguousarray(x.reshape(B, H * W, Cc))
    qkv_w = np.ascontiguousarray(np.asarray(inputs["qkv_w"], dtype=np.float32))
    qkv_b = np.ascontiguousarray(np.asarray(inputs["qkv_b"], dtype=np.float32))
    proj_w = np.ascontiguousarray(np.asarray(inputs["proj_w"], dtype=np.float32))
    proj_b = np.ascontiguousarray(np.asarray(inputs["proj_b"], dtype=np.float32))

    nc = _get_nc()
    in_maps = [
        {
            "x": xf[b],
            "qkv_w": qkv_w,
            "qkv_b": qkv_b,
            "proj_w": proj_w,
            "proj_b": proj_b,
        }
        for b in range(B)
    ]
    res = run_bass_kernel_spmd(nc, in_maps, list(range(B)), trace=trace, tmpdir=tmpdir)
    out = np.stack([res.results[b]["out"] for b in range(B)])
    return out.reshape(B, H, W, Cc).astype(np.float32), res


def kernel(x, qkv_w, qkv_b, proj_w, proj_b):
    out, _ = run(
        {
            "x": x,
            "qkv_w": qkv_w,
            "qkv_b": qkv_b,
            "proj_w": proj_w,
            "proj_b": proj_b,
        }
    )
    return out
